# revision 3
# baseline (speedup 1.0000x reference)
"""MultiHeadSelfAttentionWithRelativeBias on 8 TRN2 NeuronCores.

Sharding: data-parallel over batch (16 batches -> 2 per core).
Per-core pipeline (per batch, fully unrolled Tile program):
  - weights resident in SBUF (bf16); x^T for both batches prefetched (bf16).
  - V projection for all heads (lhsT = x^T chunk, rhs = Wv), packed into
    per-s-chunk "V_pad" tiles with a ones column per head: the ones column
    makes the attention*V matmul also emit softmax row-sums in row 64.
  - per head pair: Q^T/K^T chunks (lhsT = W chunk, rhs = x^T); Q has
    1/sqrt(D) folded into Wq on host.
  - per-head augmented tiles (bf16): qa rows 64:128 = onehot (persistent,
    loaded once), ka rows 64:128 = relative-bias features (persistent per
    head, loaded once); rows 0:64 get the fresh Q^T/K^T each batch.
  - per head, per k-chunk: one 128-contraction bf16 matmul produces
    scoresT[k,q] (QK plus bias in a single pass); exp on ScalarE -> bf16,
    then the AV matmul accumulates immediately so exp tiles recycle fast.
  - normalize: ln+exp(-x) of row 64 on ScalarE (shares one ACT table set),
    bf16 broadcast matmul (ones column x recip row), multiply on DVE.
  - O = out_all @ Wo (bf16) -> fp32 out, PSUM->SBUF copy on DVE.
"""
import numpy as np
import ml_dtypes


import concourse.bass as bass
import concourse.mybir as mybir
import concourse.tile as tile
from concourse.bass_utils import run_bass_kernel_spmd
from concourse.vector_clock import VectorClock, ScopedClock

# ---------------------------------------------------------------- constants
B, S, E, H, D = 16, 1024, 1024, 16, 64
BOARD = 32
N_CORES = 8
BPC = B // N_CORES  # batches per core
PAIRS = H // 2      # head pairs (128 partition rows per pair)
KC = E // 128       # contraction chunks
QTILES = 4          # rotating augmented-Q tiles (onehot rows persistent)
F32 = mybir.dt.float32
F32R = mybir.dt.float32r
BF16 = mybir.dt.bfloat16
AF = mybir.ActivationFunctionType

# ------------------------------------------------- walrus compat workarounds


def _patched_drain_and_barrier(self, tick_clock, wait_clock):
    gc = tick_clock.global_clock
    n = len(gc)
    for p in range(n):
        if gc[p] <= 0:
            continue
        sub = VectorClock([0] * n)
        sub.require_at_least(p, gc[p])
        d = self.nc.sync.drain()
        wait_clock.add_sem_waits(d.ins, ScopedClock({None: sub}))
    self.nc.all_engine_barrier()
    popped = self.nc._tile_sem_poison_stack.pop()
    assert popped is self._sem_poison
    self.nc.clear_and_free_semaphores(list(self.sems.allocated().values()))
    self.nc.all_engine_barrier()


tile.TileContext._drain_and_barrier = _patched_drain_and_barrier


def _split_sync_waits(nc, max_waits=1):
    """This container's walrus accepts only one sync-wait per instruction;
    move excess waits onto preceding same-engine NOPs."""
    n_split = 0
    for bb in nc.m.functions[0].blocks:
        insts = bb.instructions
        i = 0
        while i < len(insts):
            inst = insts[i]
            si = inst.sync_info
            if si is not None and si.on_wait and len(si.on_wait) > max_waits:
                waits = list(si.on_wait)
                extra, keep = waits[:-max_waits], waits[-max_waits:]
                nops = []
                for j in range(0, len(extra), max_waits):
                    nops.append(mybir.InstNoOp(
                        name=f"I-{nc.next_id()}",
                        engine=inst.engine,
                        sync_info=mybir.SyncInfo(
                            on_wait=extra[j:j + max_waits], on_update=[]),
                        bass_nofuse=True,
                    ))
                si.on_wait = keep
                inst.sync_info = si
                insts[i:i] = nops
                i += len(nops)
                n_split += 1
            i += 1
    return n_split


# ------------------------------------------------------------- build kernel


def _build_nc():
    nc = bass.Bass("TRN2", target_bir_lowering=False, debug=False,
                   num_devices=1)

    xT = nc.dram_tensor("xT", [BPC, E, S], BF16, kind="ExternalInput")
    wq = nc.dram_tensor("Wq", [E, E], BF16, kind="ExternalInput")
    wk = nc.dram_tensor("Wk", [E, E], BF16, kind="ExternalInput")
    wv = nc.dram_tensor("Wv", [E, E], BF16, kind="ExternalInput")
    wo = nc.dram_tensor("Wo", [E, E], BF16, kind="ExternalInput")
    relb = nc.dram_tensor("relb_sw", [H * 64, S], BF16, kind="ExternalInput")
    onehot = nc.dram_tensor("onehotT", [64, S], BF16, kind="ExternalInput")
    ones64 = nc.dram_tensor("ones64", [1, 64], BF16, kind="ExternalInput")
    out = nc.dram_tensor("O", [BPC, S, E], F32, kind="ExternalOutput")

    with tile.TileContext(nc) as tc:
        with (
            tc.tile_pool(name="w", bufs=32) as wp,
            tc.tile_pool(name="xt", bufs=2 * KC) as xp,
            tc.tile_pool(name="oh", bufs=1) as ohp,
            tc.tile_pool(name="qt", bufs=QTILES) as qtp,
            tc.tile_pool(name="kt", bufs=H) as ktp,
            tc.tile_pool(name="exp", bufs=6) as ep,
            tc.tile_pool(name="vpad", bufs=8) as vp,
            tc.tile_pool(name="outp", bufs=8) as outp_pool,
            tc.tile_pool(name="small", bufs=2) as sp,
            tc.tile_pool(name="osb", bufs=2) as osp,
            tc.tile_pool(name="sc_ps", bufs=2, space="PSUM") as scps,
            tc.tile_pool(name="av_ps", bufs=1, space="PSUM") as avps,
            tc.tile_pool(name="mm_ps", bufs=2, space="PSUM") as mps,
        ):
            # batch-0 x^T first: it gates the first projection matmuls
            xts = [[None] * KC for _ in range(BPC)]
            for k in range(KC):
                t = xp.tile([128, S], BF16, tag="xt", name=f"xt0_{k}",
                            bufs=2 * KC)
                nc.gpsimd.dma_start(t[:], xT.ap()[0, k * 128:(k + 1) * 128, :])
                xts[0][k] = t
            # resident weights: [e_in-chunk 128, e_out 1024] tiles
            wt = {}
            for wname, w in (("v", wv), ("q", wq), ("k", wk)):
                for k in range(KC):
                    t = wp.tile([128, E], BF16, tag="w", name=f"w{wname}{k}",
                                bufs=32)
                    nc.gpsimd.dma_start(t[:], w.ap()[k * 128:(k + 1) * 128, :])
                    wt[wname, k] = t
            # persistent augmented-K tiles: rows 64:128 = per-head rel-bias
            # features, loaded once; rows 0:64 rewritten per batch.
            ka_t = []
            for h in range(H):
                t = ktp.tile([128, S], BF16, tag="ka", name=f"ka{h}", bufs=H)
                nc.gpsimd.dma_start(t[64:128, :],
                                    relb.ap()[h * 64:(h + 1) * 64, :])
                ka_t.append(t)
            # rotating augmented-Q tiles: rows 64:128 = onehot, loaded once.
            qa_t = []
            for j in range(QTILES):
                t = qtp.tile([128, S], BF16, tag="qa", name=f"qa{j}",
                             bufs=QTILES)
                nc.gpsimd.dma_start(t[64:128, :], onehot.ap()[:, :])
                qa_t.append(t)
            ones_sb = ohp.tile([1, 64], BF16, tag="ones", name="ones_sb")
            nc.gpsimd.dma_start(ones_sb[:], ones64.ap()[:, :])
            for k in range(KC):
                t = wp.tile([128, E], BF16, tag="w", name=f"wo{k}", bufs=32)
                nc.gpsimd.dma_start(t[:], wo.ap()[k * 128:(k + 1) * 128, :])
                wt["o", k] = t
            # batch-1 x^T prefetch (last in queue: needed only mid-kernel)
            for b in range(1, BPC):
                for k in range(KC):
                    t = xp.tile([128, S], BF16, tag="xt", name=f"xt{b}_{k}",
                                bufs=2 * KC)
                    nc.gpsimd.dma_start(
                        t[:], xT.ap()[b, k * 128:(k + 1) * 128, :])
                    xts[b][k] = t

            for b in range(BPC):
                # -------- V projection for all heads: out [s-chunk, e=1024]
                vpads = []
                for sc in range(KC):
                    vt = vp.tile([128, H * 65], BF16, tag="vpad",
                                 name=f"vpad{b}_{sc}", bufs=8)
                    for n in range(2):
                        nsl = slice(n * 512, (n + 1) * 512)
                        pv = mps.tile([128, 512], F32, tag="mm_ps",
                                      name=f"vps{b}_{sc}_{n}", bufs=2)
                        for k in range(KC):
                            nc.tensor.matmul(
                                pv[:], xts[b][k][:, sc * 128:(sc + 1) * 128],
                                wt["v", k][:, nsl], start=(k == 0),
                                stop=(k == KC - 1))
                        # 8 heads per half: interleave 64 V cols + ones col
                        dst = vt[:, n * 8 * 65:(n + 1) * 8 * 65].rearrange(
                            "p (h d) -> p h d", h=8)[:, :, 0:64]
                        src = pv[:].rearrange("p (h d) -> p h d", h=8)
                        nc.vector.tensor_copy(dst, src)
                        ones_dst = vt[:, n * 8 * 65:(n + 1) * 8 * 65].rearrange(
                            "p (h d) -> p h d", h=8)[:, :, 64:65]
                        nc.vector.memset(ones_dst, 1.0)
                    vpads.append(vt)

                outps = []
                pending = []
                for m in range(PAIRS):
                    # -------- augmented Q/K tiles (bf16): rows 0:64 get the
                    # head's fresh Q^T or K^T; rows 64:128 stay persistent
                    # (onehot on the Q side, rel-bias features on the K side)
                    qa = [qa_t[(2 * m + i) % QTILES] for i in range(2)]
                    ka = [ka_t[2 * m + i] for i in range(2)]
                    for pname, dsts in (("q", qa), ("k", ka)):
                        for n in range(2):
                            nsl = slice(n * 512, (n + 1) * 512)
                            pp = mps.tile([128, 512], F32, tag="mm_ps",
                                          name=f"{pname}ps{b}_{m}_{n}",
                                          bufs=2)
                            for k in range(KC):
                                nc.tensor.matmul(
                                    pp[:],
                                    wt[pname, k][:, m * 128:(m + 1) * 128],
                                    xts[b][k][:, nsl], start=(k == 0),
                                    stop=(k == KC - 1))
                            for i in range(2):
                                nc.vector.tensor_copy(
                                    dsts[i][0:64, nsl],
                                    pp[i * 64:(i + 1) * 64, :])

                    op_t = outp_pool.tile([128, S], BF16, tag="outp",
                                          name=f"op{b}_{m}", bufs=8)
                    outps.append(op_t)

                    for h2 in range(2):
                        h = 2 * m + h2
                        avp = avps.tile([65, S], F32, tag="av_ps",
                                        name=f"av{b}_{m}_{h2}", bufs=1)
                        for kc in range(KC):
                            ksl = slice(kc * 128, (kc + 1) * 128)
                            sps = scps.tile([128, S], F32, tag="sc_ps",
                                            name=f"sps{b}_{m}_{h2}_{kc}",
                                            bufs=2)
                            for n in range(2):
                                nsl = slice(n * 512, (n + 1) * 512)
                                nc.tensor.matmul(sps[:, nsl],
                                                 ka[h2][:, ksl],
                                                 qa[h2][:, nsl],
                                                 start=True, stop=True)
                            et = ep.tile([128, S], BF16, tag="exp",
                                         name=f"exp{b}_{m}_{h2}_{kc}", bufs=6)
                            nc.scalar.activation(et[:], sps[:], AF.Exp)
                            for n in range(2):
                                nsl = slice(n * 512, (n + 1) * 512)
                                nc.tensor.matmul(
                                    avp[:, nsl],
                                    vpads[kc][:, h * 65:(h + 1) * 65],
                                    et[:, nsl], start=(kc == 0),
                                    stop=(kc == KC - 1))
                            if kc == 2 and pending:
                                pending.pop(0)()
                        # free avp fast: copy rows + ln(sums); the rest of
                        # the normalization is deferred into the next head's
                        # score loop so the PE never stalls on the ACT chain
                        un = sp.tile([64, S], BF16, tag="un",
                                     name=f"un{b}_{m}_{h2}", bufs=2)
                        nc.vector.tensor_copy(un[:], avp[0:64, :])
                        lns = sp.tile([1, S], F32, tag="lns",
                                      name=f"lns{b}_{m}_{h2}", bufs=2)
                        nc.scalar.activation(lns[:], avp[64:65, :], AF.Ln)
                        rec_r = sp.tile([1, S], BF16, tag="recr",
                                        name=f"recr{b}_{m}_{h2}", bufs=2)
                        # 1/s = exp(-ln(s)); Ln+Exp share one ACT table set
                        nc.scalar.activation(rec_r[:], lns[:], AF.Exp,
                                             scale=-1.0)

                        def _normalize(un=un, rec_r=rec_r, op_t=op_t, b=b,
                                       m=m, h2=h2):
                            for n in range(2):
                                nsl = slice(n * 512, (n + 1) * 512)
                                bps_t = mps.tile([64, 512], F32, tag="mm_ps",
                                                 name=f"bcp{b}_{m}_{h2}_{n}",
                                                 bufs=2)
                                nc.tensor.matmul(bps_t[:], ones_sb[:],
                                                 rec_r[:, nsl], start=True,
                                                 stop=True)
                                nc.vector.tensor_mul(
                                    op_t[h2 * 64:(h2 + 1) * 64, nsl],
                                    un[:, nsl], bps_t[:])

                        pending.append(_normalize)

                for fn in pending:
                    fn()
                pending = []

                # -------- output projection: O = out_all @ Wo
                for ms in range(KC):
                    msl = slice(ms * 128, (ms + 1) * 128)
                    for n in range(2):
                        nsl = slice(n * 512, (n + 1) * 512)
                        po = mps.tile([128, 512], F32, tag="mm_ps",
                                      name=f"ops{b}_{ms}_{n}", bufs=2)
                        for p in range(PAIRS):
                            nc.tensor.matmul(
                                po[:], outps[p][:, msl], wt["o", p][:, nsl],
                                start=(p == 0), stop=(p == PAIRS - 1))
                        ot = osp.tile([128, 512], F32, tag="osb",
                                      name=f"ot{b}_{ms}_{n}", bufs=2)
                        nc.vector.tensor_copy(ot[:], po[:])
                        nc.gpsimd.dma_start(out.ap()[b, msl, nsl], ot[:])

    _split_sync_waits(nc)
    return nc


_NC = None


def _get_nc():
    global _NC
    if _NC is None:
        _NC = _build_nc()
    return _NC


# ----------------------------------------------------------- host-side prep


def _host_prep(x, Wq, Wk, Wv, Wo, rel_bias):
    bf = ml_dtypes.bfloat16
    # relative-bias features: for head h, row a (a<32): rel_bias[h, j//32-a+31]
    # row 32+c: rel_bias[h, j%32-c+31]  (j = key index).
    j = np.arange(S)
    jr, jc = j // BOARD, j % BOARD
    a = np.arange(BOARD)
    relb = np.empty((H, 64, S), dtype=np.float32)
    for h in range(H):
        relb[h, 0:32, :] = rel_bias[h][jr[None, :] - a[:, None] + BOARD - 1]
        relb[h, 32:64, :] = rel_bias[h][jc[None, :] - a[:, None] + BOARD - 1]
    relb_sw = np.ascontiguousarray(relb.reshape(H * 64, S).astype(bf))

    onehot = np.zeros((64, S), dtype=np.float32)
    onehot[jr, j] = 1.0          # rows 0:32 one-hot of q//32
    onehot[32 + jc, j] = 1.0     # rows 32:64 one-hot of q%32
    onehot = np.ascontiguousarray(onehot.astype(bf))

    ones64 = np.ones((1, 64), dtype=bf)

    wq_b = np.ascontiguousarray((Wq * 0.125).astype(bf))  # fold 1/sqrt(D)
    wk_b = np.ascontiguousarray(Wk.astype(bf))
    wv_b = np.ascontiguousarray(Wv.astype(bf))
    wo_b = np.ascontiguousarray(Wo.astype(bf))

    in_maps = []
    for c in range(N_CORES):
        xc = x[c * BPC:(c + 1) * BPC]                    # [BPC, S, E]
        xt = np.ascontiguousarray(xc.transpose(0, 2, 1).astype(bf))
        in_maps.append({
            "xT": xt, "Wq": wq_b, "Wk": wk_b, "Wv": wv_b, "Wo": wo_b,
            "relb_sw": relb_sw, "onehotT": onehot, "ones64": ones64,
        })
    return in_maps


def kernel(x, Wq, Wk, Wv, Wo, rel_bias, _trace=False):
    nc = _get_nc()
    in_maps = _host_prep(np.asarray(x), np.asarray(Wq), np.asarray(Wk),
                         np.asarray(Wv), np.asarray(Wo), np.asarray(rel_bias))
    res = run_bass_kernel_spmd(nc, in_maps, core_ids=list(range(N_CORES)),
                               trace=_trace)
    out = np.concatenate([res.results[c]["O"] for c in range(N_CORES)], axis=0)
    if _trace:
        kernel.last_exec_time_ns = res.exec_time_ns
        kernel.last_results = res
    return out


# revision 9
# speedup vs baseline: 1.0878x; 1.0878x over previous
"""MultiHeadSelfAttentionWithRelativeBias on 8 TRN2 NeuronCores.

Sharding: data-parallel over batch (16 batches -> 2 per core).

Per-core pipeline, software-pipelined emission (engine queues execute in
program order, so PE-only projection groups are spliced into the
ACT-bound attention stream to fill the exp-wait bubbles):
  - weights resident in SBUF (bf16); x^T for both batches prefetched.
  - V projection for all heads packed into per-s-chunk "V_pad" tiles with
    a ones column per head: the ones column makes the attention*V matmul
    also emit softmax row-sums in row 64.
  - per head pair: Q^T/K^T chunks; Q has 1/sqrt(D) folded into Wq on host.
  - augmented bf16 tiles: qa rows 64:128 = onehot (persistent), ka rows
    64:128 = per-head rel-bias features (DMA per batch-head); rows 0:64
    get the fresh Q^T/K^T.
  - per head, per k-chunk: one 128-contraction bf16 matmul produces
    scoresT[k,q] (QK plus bias in one pass); exp on ScalarE -> bf16, AV
    matmul accumulates immediately.
  - normalize: row-sums of the pair collected into one [2,S] tile, ONE
    ln + exp(-x) per pair on ScalarE (shared ACT table set), bf16
    broadcast matmul (ones column x recip row), in-place multiply on DVE
    directly in the split (per token-half) attention-output tiles.
  - O = out_all @ Wo (bf16) -> fp32 out, PSUM->SBUF copy on DVE; O-proj
    groups of batch b are spliced into batch b+1's attention stream.
"""
import numpy as np
import ml_dtypes
from collections import deque


import concourse.bass as bass
import concourse.mybir as mybir
import concourse.tile as tile
from concourse.bass_utils import run_bass_kernel_spmd
from concourse.vector_clock import VectorClock, ScopedClock

# ---------------------------------------------------------------- constants
B, S, E, H, D = 16, 1024, 1024, 16, 64
BOARD = 32
N_CORES = 8
BPC = B // N_CORES  # batches per core
PAIRS = H // 2      # head pairs (128 partition rows per pair)
KC = E // 128       # contraction chunks
QTILES = 4          # rotating augmented-Q tiles (onehot rows persistent)
KTILES = 6          # rotating augmented-K tiles
F32 = mybir.dt.float32
BF16 = mybir.dt.bfloat16
AF = mybir.ActivationFunctionType

# ------------------------------------------------- walrus compat workarounds


def _patched_drain_and_barrier(self, tick_clock, wait_clock):
    gc = tick_clock.global_clock
    n = len(gc)
    for p in range(n):
        if gc[p] <= 0:
            continue
        sub = VectorClock([0] * n)
        sub.require_at_least(p, gc[p])
        d = self.nc.sync.drain()
        wait_clock.add_sem_waits(d.ins, ScopedClock({None: sub}))
    self.nc.all_engine_barrier()
    popped = self.nc._tile_sem_poison_stack.pop()
    assert popped is self._sem_poison
    self.nc.clear_and_free_semaphores(list(self.sems.allocated().values()))
    self.nc.all_engine_barrier()


tile.TileContext._drain_and_barrier = _patched_drain_and_barrier


def _split_sync_waits(nc, max_waits=1):
    """This container's walrus accepts only one sync-wait per instruction;
    move excess waits onto preceding same-engine NOPs."""
    n_split = 0
    for bb in nc.m.functions[0].blocks:
        insts = bb.instructions
        i = 0
        while i < len(insts):
            inst = insts[i]
            si = inst.sync_info
            if si is not None and si.on_wait and len(si.on_wait) > max_waits:
                waits = list(si.on_wait)
                extra, keep = waits[:-max_waits], waits[-max_waits:]
                nops = []
                for j in range(0, len(extra), max_waits):
                    nops.append(mybir.InstNoOp(
                        name=f"I-{nc.next_id()}",
                        engine=inst.engine,
                        sync_info=mybir.SyncInfo(
                            on_wait=extra[j:j + max_waits], on_update=[]),
                        bass_nofuse=True,
                    ))
                si.on_wait = keep
                inst.sync_info = si
                insts[i:i] = nops
                i += len(nops)
                n_split += 1
            i += 1
    return n_split


# ------------------------------------------------------------- build kernel


def _build_nc():
    nc = bass.Bass("TRN2", target_bir_lowering=False, debug=False,
                   num_devices=1)

    xT = nc.dram_tensor("xT", [BPC, E, S], BF16, kind="ExternalInput")
    wq = nc.dram_tensor("Wq", [E, E], BF16, kind="ExternalInput")
    wk = nc.dram_tensor("Wk", [E, E], BF16, kind="ExternalInput")
    wv = nc.dram_tensor("Wv", [E, E], BF16, kind="ExternalInput")
    wo = nc.dram_tensor("Wo", [E, E], BF16, kind="ExternalInput")
    relb = nc.dram_tensor("relb_sw", [H * 64, S], BF16, kind="ExternalInput")
    onehot = nc.dram_tensor("onehotT", [64, S], BF16, kind="ExternalInput")
    ones64 = nc.dram_tensor("ones64", [1, 64], BF16, kind="ExternalInput")
    out = nc.dram_tensor("O", [BPC, S, E], F32, kind="ExternalOutput")

    with tile.TileContext(nc) as tc:
        with (
            tc.tile_pool(name="w", bufs=32) as wp,
            tc.tile_pool(name="xt", bufs=2 * KC) as xp,
            tc.tile_pool(name="oh", bufs=1) as ohp,
            tc.tile_pool(name="qt", bufs=QTILES) as qtp,
            tc.tile_pool(name="kt", bufs=KTILES) as ktp,
            tc.tile_pool(name="exp", bufs=6) as ep,
            tc.tile_pool(name="vpad", bufs=16) as vp,
            tc.tile_pool(name="outp", bufs=26) as outp_pool,
            tc.tile_pool(name="small", bufs=2) as sp,
            tc.tile_pool(name="osb", bufs=2) as osp,
            tc.tile_pool(name="sc_ps", bufs=2, space="PSUM") as scps,
            tc.tile_pool(name="av_ps", bufs=1, space="PSUM") as avps,
            tc.tile_pool(name="mm_ps", bufs=2, space="PSUM") as mps,
        ):
            # batch-0 x^T first: it gates the first projection matmuls
            xts = [[None] * KC for _ in range(BPC)]
            for k in range(KC):
                t = xp.tile([128, S], BF16, tag="xt", name=f"xt0_{k}",
                            bufs=2 * KC)
                nc.gpsimd.dma_start(t[:], xT.ap()[0, k * 128:(k + 1) * 128, :])
                xts[0][k] = t
            # resident weights: [e_in-chunk 128, e_out 1024] tiles.
            # wv arrives in column halves so the first V-proj PSUM group
            # (which only reads cols 0:512 of every chunk) starts sooner.
            wt = {}
            for k in range(KC):
                t = wp.tile([128, E], BF16, tag="w", name=f"wv{k}", bufs=32)
                nc.gpsimd.dma_start(t[:, 0:512],
                                    wv.ap()[k * 128:(k + 1) * 128, 0:512])
                wt["v", k] = t
            for k in range(KC):
                nc.gpsimd.dma_start(wt["v", k][:, 512:1024],
                                    wv.ap()[k * 128:(k + 1) * 128, 512:1024])
            for wname, w in (("q", wq), ("k", wk)):
                for k in range(KC):
                    t = wp.tile([128, E], BF16, tag="w", name=f"w{wname}{k}",
                                bufs=32)
                    nc.gpsimd.dma_start(t[:], w.ap()[k * 128:(k + 1) * 128, :])
                    wt[wname, k] = t
            # rotating augmented-Q tiles: rows 64:128 = onehot, loaded once.
            qa_t = []
            for j in range(QTILES):
                t = qtp.tile([128, S], BF16, tag="qa", name=f"qa{j}",
                             bufs=QTILES)
                nc.gpsimd.dma_start(t[64:128, :], onehot.ap()[:, :])
                qa_t.append(t)
            ones_sb = ohp.tile([1, 64], BF16, tag="ones", name="ones_sb")
            nc.gpsimd.dma_start(ones_sb[:], ones64.ap()[:, :])
            for k in range(KC):
                t = wp.tile([128, E], BF16, tag="w", name=f"wo{k}", bufs=32)
                nc.gpsimd.dma_start(t[:], wo.ap()[k * 128:(k + 1) * 128, :])
                wt["o", k] = t
            # batch-1 x^T prefetch (last in queue: needed only mid-kernel)
            for b in range(1, BPC):
                for k in range(KC):
                    t = xp.tile([128, S], BF16, tag="xt", name=f"xt{b}_{k}",
                                bufs=2 * KC)
                    nc.gpsimd.dma_start(
                        t[:], xT.ap()[b, k * 128:(k + 1) * 128, :])
                    xts[b][k] = t

            # ---------------- deferred-work plumbing (software pipelining)
            fill_queue = deque()   # PE-heavy closures spliced into attention
            pending = deque()      # deferred per-pair normalize closures
            vproj_left = {}        # (b) -> # v-proj groups not yet emitted
            qk_left = {}           # (b, m) -> # qk groups not yet emitted

            def pop_fill():
                n = 2 if len(fill_queue) > 8 else 1
                for _ in range(n):
                    if fill_queue:
                        fill_queue.popleft()()

            # ---------------- phase-group builders (each returns a closure)
            vpads_all = {}

            def vproj_group(b, sc):
                def run():
                    vt = vp.tile([128, H * 65], BF16, tag="vpad",
                                 name=f"vpad{b}_{sc}", bufs=16)
                    vpads_all[b, sc] = vt
                    for n in range(2):
                        nsl = slice(n * 512, (n + 1) * 512)
                        pv = mps.tile([128, 512], F32, tag="mm_ps",
                                      name=f"vps{b}_{sc}_{n}", bufs=2)
                        for k in range(KC):
                            nc.tensor.matmul(
                                pv[:], xts[b][k][:, sc * 128:(sc + 1) * 128],
                                wt["v", k][:, nsl], start=(k == 0),
                                stop=(k == KC - 1))
                        # 8 heads per half: interleave 64 V cols + ones col
                        dst = vt[:, n * 8 * 65:(n + 1) * 8 * 65].rearrange(
                            "p (h d) -> p h d", h=8)[:, :, 0:64]
                        src = pv[:].rearrange("p (h d) -> p h d", h=8)
                        nc.vector.tensor_copy(dst, src)
                        ones_dst = vt[:, n * 8 * 65:(n + 1) * 8 * 65].rearrange(
                            "p (h d) -> p h d", h=8)[:, :, 64:65]
                        nc.vector.memset(ones_dst, 1.0)
                    vproj_left[b] -= 1
                return run

            qk_tiles = {}

            def qk_group(b, m, pname, n):
                """One QK-projection PSUM group: 8 MMs + 2 casts into the
                augmented tiles (rows 0:64)."""
                def run():
                    key = (b, m)
                    if key not in qk_tiles:
                        qa = [qa_t[(2 * m + i) % QTILES] for i in range(2)]
                        ka = []
                        for i in range(2):
                            h = 2 * m + i
                            t = ktp.tile([128, S], BF16, tag="ka",
                                         name=f"ka{b}_{m}_{i}", bufs=KTILES)
                            nc.gpsimd.dma_start(
                                t[64:128, :],
                                relb.ap()[h * 64:(h + 1) * 64, :])
                            ka.append(t)
                        qk_tiles[key] = (qa, ka)
                    qa, ka = qk_tiles[key]
                    dsts = qa if pname == "q" else ka
                    nsl = slice(n * 512, (n + 1) * 512)
                    pp = mps.tile([128, 512], F32, tag="mm_ps",
                                  name=f"{pname}ps{b}_{m}_{n}", bufs=2)
                    for k in range(KC):
                        nc.tensor.matmul(
                            pp[:], wt[pname, k][:, m * 128:(m + 1) * 128],
                            xts[b][k][:, nsl], start=(k == 0),
                            stop=(k == KC - 1))
                    for i in range(2):
                        nc.vector.tensor_copy(dsts[i][0:64, nsl],
                                              pp[i * 64:(i + 1) * 64, :])
                    qk_left[key] -= 1
                return run

            def push_qk(b, m):
                qk_left[b, m] = 4
                for pname in ("q", "k"):
                    for n in range(2):
                        fill_queue.append(qk_group(b, m, pname, n))

            outps_all = {}   # (b, m, n) -> [128, 512] bf16 attention output

            def oproj_group(b, ms, n):
                def run():
                    msl = slice((ms % 4) * 128, (ms % 4 + 1) * 128)
                    half = ms // 4
                    nsl = slice(n * 512, (n + 1) * 512)
                    po = mps.tile([128, 512], F32, tag="mm_ps",
                                  name=f"ops{b}_{ms}_{n}", bufs=2)
                    for p in range(PAIRS):
                        nc.tensor.matmul(
                            po[:], outps_all[b, p, half][:, msl],
                            wt["o", p][:, nsl],
                            start=(p == 0), stop=(p == PAIRS - 1))
                    ot = osp.tile([128, 512], F32, tag="osb",
                                  name=f"ot{b}_{ms}_{n}", bufs=2)
                    nc.vector.tensor_copy(ot[:], po[:])
                    nc.gpsimd.dma_start(
                        out.ap()[b, ms * 128:(ms + 1) * 128, nsl], ot[:])
                return run

            # ---------------- main software-pipelined schedule
            for b in range(BPC):
                if b == 0:
                    vproj_left[0] = KC
                    for sc in range(KC):
                        vproj_group(0, sc)()
                    push_qk(0, 0)

                for m in range(PAIRS):
                    # correctness force-drains: this pair's QK projection
                    # (and, at batch start, this batch's V projection) must
                    # be fully emitted before its attention reads the tiles
                    while vproj_left[b] > 0 or qk_left[b, m] > 0:
                        pop_fill()
                    # feed the fill queue: QK-proj of the next pair; late in
                    # the batch, the next batch's V-projection
                    if m + 1 < PAIRS:
                        push_qk(b, m + 1)
                    elif b + 1 < BPC:
                        push_qk(b + 1, 0)
                    if b + 1 < BPC and m in (5, 6):
                        if m == 5:
                            vproj_left[b + 1] = KC
                        for sc in range(4 * (m - 5), 4 * (m - 5) + 4):
                            fill_queue.append(vproj_group(b + 1, sc))

                    opn = []
                    for n in range(2):
                        t = outp_pool.tile([128, 512], BF16, tag="outp",
                                           name=f"op{b}_{m}_{n}", bufs=26)
                        outps_all[b, m, n] = t
                        opn.append(t)

                    qa, ka = qk_tiles[b, m]
                    rec_r = []
                    for h2 in range(2):
                        h = 2 * m + h2
                        avp = avps.tile([65, S], F32, tag="av_ps",
                                        name=f"av{b}_{m}_{h2}", bufs=1)
                        for kc in range(KC):
                            ksl = slice(kc * 128, (kc + 1) * 128)
                            sps = scps.tile([128, S], F32, tag="sc_ps",
                                            name=f"sps{b}_{m}_{h2}_{kc}",
                                            bufs=2)
                            for n in range(2):
                                nsl = slice(n * 512, (n + 1) * 512)
                                nc.tensor.matmul(sps[:, nsl],
                                                 ka[h2][:, ksl],
                                                 qa[h2][:, nsl],
                                                 start=True, stop=True)
                            et = ep.tile([128, S], BF16, tag="exp",
                                         name=f"exp{b}_{m}_{h2}_{kc}", bufs=6)
                            nc.scalar.activation(et[:], sps[:], AF.Exp)
                            for n in range(2):
                                nsl = slice(n * 512, (n + 1) * 512)
                                nc.tensor.matmul(
                                    avp[:, nsl],
                                    vpads_all[b, kc][:, h * 65:(h + 1) * 65],
                                    et[:, nsl], start=(kc == 0),
                                    stop=(kc == KC - 1))
                            # splice deferred PE work into the exp bubbles
                            if kc % 2 == 1:
                                pop_fill()
                            elif kc == 6 and pending:
                                pending.popleft()()
                        # free avp fast: rows straight into the output tiles
                        # (normalized in place later); 1/s = exp(-ln(s))
                        # (Ln+Exp share one ACT table set, base 64 is legal)
                        for n in range(2):
                            nsl = slice(n * 512, (n + 1) * 512)
                            nc.vector.tensor_copy(
                                opn[n][h2 * 64:(h2 + 1) * 64, :],
                                avp[0:64, nsl])
                        lns = sp.tile([1, S], F32, tag="lns",
                                      name=f"lns{b}_{m}_{h2}", bufs=2)
                        nc.scalar.activation(lns[:], avp[64:65, :], AF.Ln)
                        r = sp.tile([1, S], BF16, tag="recr",
                                    name=f"recr{b}_{m}_{h2}", bufs=3)
                        nc.scalar.activation(r[:], lns[:], AF.Exp,
                                             scale=-1.0)
                        rec_r.append(r)

                    def _normalize(opn=opn, rec_r=rec_r, b=b, m=m):
                        for h2 in range(2):
                            for n in range(2):
                                nsl = slice(n * 512, (n + 1) * 512)
                                bps_t = mps.tile([64, 512], F32, tag="mm_ps",
                                                 name=f"bcp{b}_{m}_{h2}_{n}",
                                                 bufs=2)
                                nc.tensor.matmul(bps_t[:], ones_sb[:],
                                                 rec_r[h2][:, nsl],
                                                 start=True, stop=True)
                                rows = slice(h2 * 64, (h2 + 1) * 64)
                                nc.vector.tensor_mul(
                                    opn[n][rows, :], opn[n][rows, :],
                                    bps_t[:])

                    pending.append(_normalize)

                # flush any leftover normalizes at batch end
                while pending:
                    pending.popleft()()
                # queue O-projection of this batch; for the last batch emit
                # directly (nothing left to overlap with)
                for ms in range(KC):
                    for n in range(2):
                        if b + 1 < BPC:
                            fill_queue.append(oproj_group(b, ms, n))
                        else:
                            oproj_group(b, ms, n)()
            while fill_queue:
                pop_fill()

    _split_sync_waits(nc)
    return nc


_NC = None


def _get_nc():
    global _NC
    if _NC is None:
        _NC = _build_nc()
    return _NC


# ----------------------------------------------------------- host-side prep


def _host_prep(x, Wq, Wk, Wv, Wo, rel_bias):
    bf = ml_dtypes.bfloat16
    # relative-bias features: for head h, row a (a<32): rel_bias[h, j//32-a+31]
    # row 32+c: rel_bias[h, j%32-c+31]  (j = key index).
    j = np.arange(S)
    jr, jc = j // BOARD, j % BOARD
    a = np.arange(BOARD)
    relb = np.empty((H, 64, S), dtype=np.float32)
    for h in range(H):
        relb[h, 0:32, :] = rel_bias[h][jr[None, :] - a[:, None] + BOARD - 1]
        relb[h, 32:64, :] = rel_bias[h][jc[None, :] - a[:, None] + BOARD - 1]
    relb_sw = np.ascontiguousarray(relb.reshape(H * 64, S).astype(bf))

    onehot = np.zeros((64, S), dtype=np.float32)
    onehot[jr, j] = 1.0          # rows 0:32 one-hot of q//32
    onehot[32 + jc, j] = 1.0     # rows 32:64 one-hot of q%32
    onehot = np.ascontiguousarray(onehot.astype(bf))

    ones64 = np.ones((1, 64), dtype=bf)

    wq_b = np.ascontiguousarray((Wq * 0.125).astype(bf))  # fold 1/sqrt(D)
    wk_b = np.ascontiguousarray(Wk.astype(bf))
    wv_b = np.ascontiguousarray(Wv.astype(bf))
    wo_b = np.ascontiguousarray(Wo.astype(bf))

    in_maps = []
    for c in range(N_CORES):
        xc = x[c * BPC:(c + 1) * BPC]                    # [BPC, S, E]
        xt = np.ascontiguousarray(xc.transpose(0, 2, 1).astype(bf))
        in_maps.append({
            "xT": xt, "Wq": wq_b, "Wk": wk_b, "Wv": wv_b, "Wo": wo_b,
            "relb_sw": relb_sw, "onehotT": onehot, "ones64": ones64,
        })
    return in_maps


def kernel(x, Wq, Wk, Wv, Wo, rel_bias, _trace=False):
    nc = _get_nc()
    in_maps = _host_prep(np.asarray(x), np.asarray(Wq), np.asarray(Wk),
                         np.asarray(Wv), np.asarray(Wo), np.asarray(rel_bias))
    res = run_bass_kernel_spmd(nc, in_maps, core_ids=list(range(N_CORES)),
                               trace=_trace)
    out = np.concatenate([res.results[c]["O"] for c in range(N_CORES)], axis=0)
    if _trace:
        kernel.last_exec_time_ns = res.exec_time_ns
        kernel.last_results = res
    return out


# revision 20
# speedup vs baseline: 1.1265x; 1.0355x over previous
"""MultiHeadSelfAttentionWithRelativeBias on 8 TRN2 NeuronCores.

Sharding: data-parallel over batch (16 batches -> 2 per core).

Per-core pipeline, software-pipelined emission (engine queues execute in
program order, so PE-only projection groups are spliced into the
ACT-bound attention stream to fill the exp-wait bubbles):
  - weights resident in SBUF (bf16); x^T for both batches prefetched.
  - V projection for all heads packed into per-s-chunk "V_pad" tiles with
    a ones column per head: the ones column makes the attention*V matmul
    also emit softmax row-sums in row 64.
  - per head pair: Q^T/K^T chunks; Q has 1/sqrt(D) folded into Wq on host.
  - augmented bf16 tiles: qa rows 64:128 = onehot (persistent), ka rows
    64:128 = per-head rel-bias features (DMA per batch-head); rows 0:64
    get the fresh Q^T/K^T.
  - per head, per k-chunk: one 128-contraction bf16 matmul produces
    scoresT[k,q] (QK plus bias in one pass); exp on ScalarE -> bf16, AV
    matmul accumulates immediately.
  - normalize: row-sums of the pair collected into one [2,S] tile, ONE
    ln + exp(-x) per pair on ScalarE (shared ACT table set), bf16
    broadcast matmul (ones column x recip row), in-place multiply on DVE
    directly in the split (per token-half) attention-output tiles.
  - O = out_all @ Wo (bf16) -> fp32 out, PSUM->SBUF copy on DVE; O-proj
    groups of batch b are spliced into batch b+1's attention stream.
"""
import numpy as np
import ml_dtypes
from collections import deque


import concourse.bass as bass
import concourse.mybir as mybir
import concourse.tile as tile
from concourse.bass_utils import run_bass_kernel_spmd
from concourse.vector_clock import VectorClock, ScopedClock

# ---------------------------------------------------------------- constants
B, S, E, H, D = 16, 1024, 1024, 16, 64
BOARD = 32
N_CORES = 8
BPC = B // N_CORES  # batches per core
PAIRS = H // 2      # head pairs (128 partition rows per pair)
KC = E // 128       # contraction chunks
QTILES = 4          # rotating augmented-Q tiles (onehot rows persistent)
KTILES = 6          # rotating augmented-K tiles
F32 = mybir.dt.float32
F32R = mybir.dt.float32r
BF16 = mybir.dt.bfloat16
AF = mybir.ActivationFunctionType

# ------------------------------------------------- walrus compat workarounds


def _patched_drain_and_barrier(self, tick_clock, wait_clock):
    gc = tick_clock.global_clock
    n = len(gc)
    for p in range(n):
        if gc[p] <= 0:
            continue
        sub = VectorClock([0] * n)
        sub.require_at_least(p, gc[p])
        d = self.nc.sync.drain()
        wait_clock.add_sem_waits(d.ins, ScopedClock({None: sub}))
    self.nc.all_engine_barrier()
    popped = self.nc._tile_sem_poison_stack.pop()
    assert popped is self._sem_poison
    self.nc.clear_and_free_semaphores(list(self.sems.allocated().values()))
    self.nc.all_engine_barrier()


tile.TileContext._drain_and_barrier = _patched_drain_and_barrier


def _split_sync_waits(nc, max_waits=1):
    """This container's walrus accepts only one sync-wait per instruction;
    move excess waits onto preceding same-engine NOPs."""
    n_split = 0
    for bb in nc.m.functions[0].blocks:
        insts = bb.instructions
        i = 0
        while i < len(insts):
            inst = insts[i]
            si = inst.sync_info
            if si is not None and si.on_wait and len(si.on_wait) > max_waits:
                waits = list(si.on_wait)
                extra, keep = waits[:-max_waits], waits[-max_waits:]
                nops = []
                for j in range(0, len(extra), max_waits):
                    nops.append(mybir.InstNoOp(
                        name=f"I-{nc.next_id()}",
                        engine=inst.engine,
                        sync_info=mybir.SyncInfo(
                            on_wait=extra[j:j + max_waits], on_update=[]),
                        bass_nofuse=True,
                    ))
                si.on_wait = keep
                inst.sync_info = si
                insts[i:i] = nops
                i += len(nops)
                n_split += 1
            i += 1
    return n_split


# ------------------------------------------------------------- build kernel


def _build_nc():
    nc = bass.Bass("TRN2", target_bir_lowering=False, debug=False,
                   num_devices=1)

    xT = nc.dram_tensor("xT", [BPC, E, S], BF16, kind="ExternalInput")
    wq = nc.dram_tensor("Wq", [E, E], BF16, kind="ExternalInput")
    wk = nc.dram_tensor("Wk", [E, E], BF16, kind="ExternalInput")
    wv = nc.dram_tensor("Wv", [E, E], BF16, kind="ExternalInput")
    wo = nc.dram_tensor("Wo", [E, E], BF16, kind="ExternalInput")
    relb = nc.dram_tensor("relb_sw", [H * 64, S], BF16, kind="ExternalInput")
    onehot = nc.dram_tensor("onehotT", [64, S], BF16, kind="ExternalInput")
    ones64 = nc.dram_tensor("ones64", [1, 64], BF16, kind="ExternalInput")
    out = nc.dram_tensor("O", [BPC, S, E], F32, kind="ExternalOutput")

    with tile.TileContext(nc) as tc:
        with (
            tc.tile_pool(name="w", bufs=32) as wp,
            tc.tile_pool(name="xt", bufs=2 * KC) as xp,
            tc.tile_pool(name="oh", bufs=1) as ohp,
            tc.tile_pool(name="qt", bufs=QTILES) as qtp,
            tc.tile_pool(name="kt", bufs=KTILES) as ktp,
            tc.tile_pool(name="exp", bufs=6) as ep,
            tc.tile_pool(name="vpad", bufs=16) as vp,
            tc.tile_pool(name="outp", bufs=26) as outp_pool,
            tc.tile_pool(name="small", bufs=2) as sp,
            tc.tile_pool(name="osb", bufs=2) as osp,
            tc.tile_pool(name="sc_ps", bufs=2, space="PSUM") as scps,
            tc.tile_pool(name="av_ps", bufs=1, space="PSUM") as avps,
            tc.tile_pool(name="mm_ps", bufs=2, space="PSUM") as mps,
        ):
            # batch-0 x^T first: it gates the first projection matmuls
            xts = [[None] * KC for _ in range(BPC)]
            for k in range(KC):
                t = xp.tile([128, S], BF16, tag="xt", name=f"xt0_{k}",
                            bufs=2 * KC)
                nc.gpsimd.dma_start(t[:], xT.ap()[0, k * 128:(k + 1) * 128, :])
                xts[0][k] = t
            # resident weights: [e_in-chunk 128, e_out 1024] tiles.
            # wv arrives in column halves so the first V-proj PSUM group
            # (which only reads cols 0:512 of every chunk) starts sooner.
            wt = {}
            for k in range(KC):
                t = wp.tile([128, E], BF16, tag="w", name=f"wv{k}", bufs=32)
                nc.gpsimd.dma_start(t[:, 0:512],
                                    wv.ap()[k * 128:(k + 1) * 128, 0:512])
                wt["v", k] = t
            for k in range(KC):
                nc.gpsimd.dma_start(wt["v", k][:, 512:1024],
                                    wv.ap()[k * 128:(k + 1) * 128, 512:1024])
            for wname, w in (("q", wq), ("k", wk)):
                for k in range(KC):
                    t = wp.tile([128, E], BF16, tag="w", name=f"w{wname}{k}",
                                bufs=32)
                    nc.gpsimd.dma_start(t[:], w.ap()[k * 128:(k + 1) * 128, :])
                    wt[wname, k] = t
            # rotating augmented-Q tiles: rows 64:128 = onehot, loaded once.
            qa_t = []
            for j in range(QTILES):
                t = qtp.tile([128, S], BF16, tag="qa", name=f"qa{j}",
                             bufs=QTILES)
                nc.gpsimd.dma_start(t[64:128, :], onehot.ap()[:, :])
                qa_t.append(t)
            ones_sb = ohp.tile([1, 64], BF16, tag="ones", name="ones_sb")
            nc.gpsimd.dma_start(ones_sb[:], ones64.ap()[:, :])
            for k in range(KC):
                t = wp.tile([128, E], BF16, tag="w", name=f"wo{k}", bufs=32)
                nc.gpsimd.dma_start(t[:], wo.ap()[k * 128:(k + 1) * 128, :])
                wt["o", k] = t
            # batch-1 x^T prefetch (last in queue: needed only mid-kernel)
            for b in range(1, BPC):
                for k in range(KC):
                    t = xp.tile([128, S], BF16, tag="xt", name=f"xt{b}_{k}",
                                bufs=2 * KC)
                    nc.gpsimd.dma_start(
                        t[:], xT.ap()[b, k * 128:(k + 1) * 128, :])
                    xts[b][k] = t

            # ---------------- deferred-work plumbing (software pipelining)
            fill_queue = deque()   # PE-heavy closures spliced into attention
            pending = deque()      # deferred per-pair normalize closures
            oproj_late = []        # held-back O-proj groups (token half 1)
            vproj_left = {}        # (b) -> # v-proj groups not yet emitted
            qk_left = {}           # (b, m) -> # qk groups not yet emitted

            def pop_fill():
                n = 2 if len(fill_queue) > 8 else 1
                for _ in range(n):
                    if fill_queue:
                        fill_queue.popleft()()

            # ---------------- phase-group builders (each returns a closure)
            vpads_all = {}

            def vproj_group(b, sc):
                def run():
                    vt = vp.tile([128, H * 65], BF16, tag="vpad",
                                 name=f"vpad{b}_{sc}", bufs=16)
                    vpads_all[b, sc] = vt
                    for n in range(2):
                        nsl = slice(n * 512, (n + 1) * 512)
                        pv = mps.tile([128, 512], F32, tag="mm_ps",
                                      name=f"vps{b}_{sc}_{n}", bufs=2)
                        for k in range(KC):
                            nc.tensor.matmul(
                                pv[:], xts[b][k][:, sc * 128:(sc + 1) * 128],
                                wt["v", k][:, nsl], start=(k == 0),
                                stop=(k == KC - 1))
                        # 8 heads per half: interleave 64 V cols + ones col
                        dst = vt[:, n * 8 * 65:(n + 1) * 8 * 65].rearrange(
                            "p (h d) -> p h d", h=8)[:, :, 0:64]
                        src = pv[:].rearrange("p (h d) -> p h d", h=8)
                        nc.vector.tensor_copy(dst, src)
                        ones_dst = vt[:, n * 8 * 65:(n + 1) * 8 * 65].rearrange(
                            "p (h d) -> p h d", h=8)[:, :, 64:65]
                        nc.vector.memset(ones_dst, 1.0)
                    vproj_left[b] -= 1
                return run

            qk_tiles = {}

            def qk_group(b, m, pname, n):
                """One QK-projection PSUM group: 8 MMs + 2 casts into the
                augmented tiles (rows 0:64)."""
                def run():
                    key = (b, m)
                    if key not in qk_tiles:
                        qa = [qa_t[(2 * m + i) % QTILES] for i in range(2)]
                        ka = []
                        for i in range(2):
                            h = 2 * m + i
                            t = ktp.tile([128, S], BF16, tag="ka",
                                         name=f"ka{b}_{m}_{i}", bufs=KTILES)
                            nc.gpsimd.dma_start(
                                t[64:128, :],
                                relb.ap()[h * 64:(h + 1) * 64, :])
                            ka.append(t)
                        qk_tiles[key] = (qa, ka)
                    qa, ka = qk_tiles[key]
                    dsts = qa if pname == "q" else ka
                    nsl = slice(n * 512, (n + 1) * 512)
                    pp = mps.tile([128, 512], F32, tag="mm_ps",
                                  name=f"{pname}ps{b}_{m}_{n}", bufs=2)
                    for k in range(KC):
                        nc.tensor.matmul(
                            pp[:], wt[pname, k][:, m * 128:(m + 1) * 128],
                            xts[b][k][:, nsl], start=(k == 0),
                            stop=(k == KC - 1))
                    for i in range(2):
                        nc.vector.tensor_copy(dsts[i][0:64, nsl],
                                              pp[i * 64:(i + 1) * 64, :])
                    qk_left[key] -= 1
                return run

            def push_qk(b, m):
                qk_left[b, m] = 4
                for pname in ("q", "k"):
                    for n in range(2):
                        fill_queue.append(qk_group(b, m, pname, n))

            outps_all = {}   # (b, m, n) -> [128, 512] bf16 attention output

            def oproj_group(b, ms, n):
                def run():
                    msl = slice((ms % 4) * 128, (ms % 4 + 1) * 128)
                    half = ms // 4
                    nsl = slice(n * 512, (n + 1) * 512)
                    po = mps.tile([128, 512], F32, tag="mm_ps",
                                  name=f"ops{b}_{ms}_{n}", bufs=2)
                    for p in range(PAIRS):
                        nc.tensor.matmul(
                            po[:], outps_all[b, p, half][:, msl],
                            wt["o", p][:, nsl],
                            start=(p == 0), stop=(p == PAIRS - 1))
                    ot = osp.tile([128, 512], F32, tag="osb",
                                  name=f"ot{b}_{ms}_{n}", bufs=2)
                    nc.vector.tensor_copy(ot[:], po[:])
                    nc.gpsimd.dma_start(
                        out.ap()[b, ms * 128:(ms + 1) * 128, nsl], ot[:])
                return run

            # ---------------- main software-pipelined schedule
            for b in range(BPC):
                if b == 0:
                    vproj_left[0] = KC
                    for sc in range(KC):
                        vproj_group(0, sc)()
                    push_qk(0, 0)

                for m in range(PAIRS):
                    # correctness force-drains: this pair's QK projection
                    # (and, at batch start, this batch's V projection) must
                    # be fully emitted before its attention reads the tiles
                    while vproj_left[b] > 0 or qk_left[b, m] > 0:
                        pop_fill()
                    # feed the fill queue: QK-proj of the next pair; late in
                    # the batch, the next batch's V-projection
                    if m + 1 < PAIRS:
                        push_qk(b, m + 1)
                    elif b + 1 < BPC:
                        push_qk(b + 1, 0)
                    if b + 1 < BPC and m in (5, 6):
                        if m == 5:
                            vproj_left[b + 1] = KC
                        for sc in range(4 * (m - 5), 4 * (m - 5) + 4):
                            fill_queue.append(vproj_group(b + 1, sc))
                    # previous batch's token-half-1 O-projection feeds the
                    # late pairs (which otherwise run out of fill work)
                    if m == 4 and oproj_late:
                        fill_queue.extend(oproj_late)
                        oproj_late.clear()

                    opn = []
                    for n in range(2):
                        t = outp_pool.tile([128, 512], BF16, tag="outp",
                                           name=f"op{b}_{m}_{n}", bufs=26)
                        outps_all[b, m, n] = t
                        opn.append(t)

                    qa, ka = qk_tiles[b, m]
                    rec_r = []
                    for h2 in range(2):
                        h = 2 * m + h2
                        avp = avps.tile([65, S], F32, tag="av_ps",
                                        name=f"av{b}_{m}_{h2}", bufs=1)
                        # scores(kc) are emitted one chunk ahead of AV(kc)
                        # so the PE never sits on exp(kc)'s latency
                        ets = {}
                        for kc in range(KC + 1):
                            if kc < KC:
                                ksl = slice(kc * 128, (kc + 1) * 128)
                                sps = scps.tile([128, S], F32, tag="sc_ps",
                                                name=f"sps{b}_{m}_{h2}_{kc}",
                                                bufs=2)
                                for n in range(2):
                                    nsl = slice(n * 512, (n + 1) * 512)
                                    nc.tensor.matmul(sps[:, nsl],
                                                     ka[h2][:, ksl],
                                                     qa[h2][:, nsl],
                                                     start=True, stop=True)
                                et = ep.tile([128, S], BF16, tag="exp",
                                             name=f"exp{b}_{m}_{h2}_{kc}",
                                             bufs=6)
                                nc.scalar.activation(et[:], sps[:], AF.Exp)
                                ets[kc] = et
                            if kc > 0:
                                av_kc = kc - 1
                                et_av = ets.pop(av_kc)
                                for n in range(2):
                                    nsl = slice(n * 512, (n + 1) * 512)
                                    nc.tensor.matmul(
                                        avp[:, nsl],
                                        vpads_all[b, av_kc][
                                            :, h * 65:(h + 1) * 65],
                                        et_av[:, nsl],
                                        start=(av_kc == 0),
                                        stop=(av_kc == KC - 1))
                            # splice deferred PE work into the exp bubbles
                            if kc % 2 == 1:
                                pop_fill()
                            elif kc == 6 and pending:
                                pending.popleft()()
                        # free avp fast: rows straight into the output tiles
                        # (normalized in place later); 1/s on DVE (keeps the
                        # reciprocal off the exp-saturated ScalarE queue),
                        # bitcast to f32r for the broadcast-matmul rhs
                        for n in range(2):
                            nsl = slice(n * 512, (n + 1) * 512)
                            nc.vector.tensor_copy(
                                opn[n][h2 * 64:(h2 + 1) * 64, :],
                                avp[0:64, nsl])
                        lns = sp.tile([1, S], F32, tag="lns",
                                      name=f"lns{b}_{m}_{h2}", bufs=2)
                        nc.scalar.activation(lns[:], avp[64:65, :], AF.Ln)
                        r = sp.tile([1, S], BF16, tag="recr",
                                    name=f"recr{b}_{m}_{h2}", bufs=3)
                        nc.scalar.activation(r[:], lns[:], AF.Exp,
                                             scale=-1.0)
                        rec_r.append(r)

                    def _normalize(opn=opn, rec_r=rec_r, b=b, m=m):
                        # both heads' recip rows broadcast into one PSUM
                        # tile (partition bases 0 and 64), one full-height
                        # multiply per token half
                        for n in range(2):
                            nsl = slice(n * 512, (n + 1) * 512)
                            bps_t = mps.tile([128, 512], F32, tag="mm_ps",
                                             name=f"bcp{b}_{m}_{n}", bufs=2)
                            for h2 in range(2):
                                nc.tensor.matmul(
                                    bps_t[h2 * 64:(h2 + 1) * 64, :],
                                    ones_sb[:], rec_r[h2][:, nsl],
                                    start=True, stop=True)
                            nc.vector.tensor_mul(opn[n][:, :], opn[n][:, :],
                                                 bps_t[:])

                    pending.append(_normalize)

                # flush any leftover normalizes at batch end
                while pending:
                    pending.popleft()()
                # queue O-projection of this batch: token-half 0 drains at
                # the start of the next batch (freeing output-tile slots),
                # token-half 1 is held back for its late pairs; for the
                # last batch emit directly (nothing left to overlap with)
                for ms in range(KC):
                    for n in range(2):
                        if b + 1 >= BPC:
                            oproj_group(b, ms, n)()
                        elif ms < 4:
                            fill_queue.append(oproj_group(b, ms, n))
                        else:
                            oproj_late.append(oproj_group(b, ms, n))
            while fill_queue:
                pop_fill()

    _split_sync_waits(nc)
    return nc


_NC = None


def _get_nc():
    global _NC
    if _NC is None:
        _NC = _build_nc()
    return _NC


# ----------------------------------------------------------- host-side prep


def _host_prep(x, Wq, Wk, Wv, Wo, rel_bias):
    bf = ml_dtypes.bfloat16
    # relative-bias features: for head h, row a (a<32): rel_bias[h, j//32-a+31]
    # row 32+c: rel_bias[h, j%32-c+31]  (j = key index).
    j = np.arange(S)
    jr, jc = j // BOARD, j % BOARD
    a = np.arange(BOARD)
    relb = np.empty((H, 64, S), dtype=np.float32)
    for h in range(H):
        relb[h, 0:32, :] = rel_bias[h][jr[None, :] - a[:, None] + BOARD - 1]
        relb[h, 32:64, :] = rel_bias[h][jc[None, :] - a[:, None] + BOARD - 1]
    relb_sw = np.ascontiguousarray(relb.reshape(H * 64, S).astype(bf))

    onehot = np.zeros((64, S), dtype=np.float32)
    onehot[jr, j] = 1.0          # rows 0:32 one-hot of q//32
    onehot[32 + jc, j] = 1.0     # rows 32:64 one-hot of q%32
    onehot = np.ascontiguousarray(onehot.astype(bf))

    ones64 = np.ones((1, 64), dtype=bf)

    wq_b = np.ascontiguousarray((Wq * 0.125).astype(bf))  # fold 1/sqrt(D)
    wk_b = np.ascontiguousarray(Wk.astype(bf))
    wv_b = np.ascontiguousarray(Wv.astype(bf))
    wo_b = np.ascontiguousarray(Wo.astype(bf))

    in_maps = []
    for c in range(N_CORES):
        xc = x[c * BPC:(c + 1) * BPC]                    # [BPC, S, E]
        xt = np.ascontiguousarray(xc.transpose(0, 2, 1).astype(bf))
        in_maps.append({
            "xT": xt, "Wq": wq_b, "Wk": wk_b, "Wv": wv_b, "Wo": wo_b,
            "relb_sw": relb_sw, "onehotT": onehot, "ones64": ones64,
        })
    return in_maps


def kernel(x, Wq, Wk, Wv, Wo, rel_bias, _trace=False):
    nc = _get_nc()
    in_maps = _host_prep(np.asarray(x), np.asarray(Wq), np.asarray(Wk),
                         np.asarray(Wv), np.asarray(Wo), np.asarray(rel_bias))
    res = run_bass_kernel_spmd(nc, in_maps, core_ids=list(range(N_CORES)),
                               trace=_trace)
    out = np.concatenate([res.results[c]["O"] for c in range(N_CORES)], axis=0)
    if _trace:
        kernel.last_exec_time_ns = res.exec_time_ns
        kernel.last_results = res
    return out
